# revision 1
# baseline (speedup 1.0000x reference)
"""Causal self-attention block (qkv proj + 16-head causal attention + out_proj
+ c_proj) on 8 trn2 NeuronCores, data-parallel over the batch (B=8: one batch
element per core).

Layout strategy (per core, batch element b):
  - Activations are kept feature-major [feature, token] on chip so every
    linear layer is a plain   out = W_T.T @ act   matmul chain with the
    (host-pre-transposed) weight as the stationary operand. No on-device
    transposes at all.
  - Attention computes transposed scores  sT[tk, tq] = k_h.T q_h  per head
    pair (row-tiled K=64 matmuls run concurrently on the PE), exp with no
    max-subtraction (scores here are bounded by a few units), causal mask
    accumulated into the scores psum by a bf16 identity-matmul, and the AV
    product consumes sT directly with token-major V tiles as the stationary
    operand. A fused ones-row in the V operand (M=65) yields the softmax
    denominator for free; batched reciprocals and K=16 indicator matmuls
    broadcast 1/denom across partitions for the normalization.
  - All matmuls run in float32r (TF32-like, ~1e-4 rel precision, 4x the
    throughput of fp32 on the PE).
"""

import sys

if "/opt/trn_rl_repo" not in sys.path:
    sys.path.insert(0, "/opt/trn_rl_repo")

import ml_dtypes
import numpy as np

import concourse.bass as bass  # noqa: F401  (bass types used via tile/bacc)
import concourse.tile as tile
from concourse import bacc, mybir
from concourse.bass_utils import run_bass_kernel_spmd

B, T, E, H = 8, 1024, 1024, 16
DH = E // H          # 64
JQK = 2 * E          # q+k fused feature dim (2048)
F32 = mybir.dt.float32
F32R = mybir.dt.float32r
BF16 = mybir.dt.bfloat16
Act = mybir.ActivationFunctionType

TRACE = False        # test harness flips this for profiled runs
PHASE_LIMIT = 4      # debug: 1=qk proj, 2=+v, 3=+attention, 4=full
_CACHE = {}


def _emit(nc, tc, aps):
    (xT, wqkT, wvT, bqk, bvrow, woutT, bout, wcT, bc, mask01, ones,
     onesbf, outT) = aps
    ET = E // 128     # 8  e-tiles (contraction)
    TT = T // 128     # 8  token tiles
    NT = T // 512     # 2  512-wide token column chunks

    # The kernel is emitted as one software-pipelined stream so the in-order
    # PE queue always has dense matmul work while ACT paces the attention
    # exps:
    #   S1: qk features for pairs 0-3 (j-groups 0,2) + V heads 0-7
    #   S2: attention (c0+c1, pairs 0-3) interleaved with qk j-groups 1,3 +
    #       V heads 8-15
    #   S3: attention (c0, pairs 4-7); normalize c0; attention (c1, pairs
    #       4-7) interleaved with out_proj on token-chunk 0
    #   S4: normalize c1; out_proj chunk 1; c_proj
    # Pool stack (LIFO): den/y/qk/v persist; x/wqk and the S2 attention pools
    # release at the S2/S3 boundary where w3/z and the S3 attention pools
    # open in their space.
    consts = tc.alloc_tile_pool(name="consts", bufs=1)
    onesb = consts.tile([128, 128], F32R, tag="onesb")
    mask01b = consts.tile([128, 128], BF16, tag="mask01b")
    bqkb = consts.tile([128, JQK // 128], F32, tag="bqkb")
    bvb = consts.tile([1, E], F32R, tag="bvb")
    boutb = consts.tile([128, E // 128], F32, tag="boutb")
    bcb = consts.tile([128, E // 128], F32, tag="bcb")
    nc.sync.dma_start(out=onesb, in_=ones)
    nc.sync.dma_start(out=mask01b, in_=mask01)
    nc.sync.dma_start(out=bqkb, in_=bqk)
    nc.sync.dma_start(out=bvb[0:1, :], in_=bvrow)
    nc.sync.dma_start(out=boutb, in_=bout)
    nc.sync.dma_start(out=bcb, in_=bc)

    psum = tc.alloc_tile_pool(name="psum", bufs=1, space="PSUM")
    p_dram = tc.alloc_tile_pool(name="p_dram", bufs=1, space="DRAM")
    p_den = tc.alloc_tile_pool(name="p_den", bufs=1)
    p_y = tc.alloc_tile_pool(name="p_y", bufs=1)
    p_qk = tc.alloc_tile_pool(name="p_qk", bufs=1)
    p_v = tc.alloc_tile_pool(name="p_v", bufs=1)
    p_x = tc.alloc_tile_pool(name="p_x", bufs=1)
    p_wqk = tc.alloc_tile_pool(name="p_wqk", bufs=16)
    dden = p_dram.tile([64, 512], F32, tag="dden")
    denall = p_den.tile([64, 512], F32, tag="denall")
    recall = p_den.tile([64, 512], F32, tag="recall")
    yt = p_y.tile([128, ET, T], F32R)
    qkt = p_qk.tile([128, JQK // 128, T], F32R)
    vt = p_v.tile([128, TT, H, DH + 1], BF16)
    xt = p_x.tile([128, ET, T], F32R)

    def mm_psum(tag):
        return psum.tile([128, 512], F32, tag=tag, bufs=2, name="ps_" + tag)

    # ---- dense generators: qkv projection ---------------------------------
    def qk_gen(jg):
        """qkT[j, t] = Wqk x^T + bqk for the 512-wide feature group jg."""
        wtiles = []
        for et in range(ET):
            if jg == 0:                    # interleave x loads with group 0
                nc.sync.dma_start(out=xt[:, et, :],
                                  in_=xT[et * 128:(et + 1) * 128, :])
            wt = p_wqk.tile([128, 512], F32R, tag="wqk", name="wt")
            nc.sync.dma_start(out=wt, in_=wqkT[et * 128:(et + 1) * 128,
                                              jg * 512:(jg + 1) * 512])
            wtiles.append(wt)
        for js in range(4):
            jt = jg * 4 + js
            for th in range(NT):
                ps = mm_psum("mm")
                for et in range(ET):
                    nc.tensor.matmul(
                        ps,
                        wtiles[et][:, js * 128:(js + 1) * 128],
                        xt[:, et, th * 512:(th + 1) * 512],
                        start=(et == 0), stop=(et == ET - 1))
                    yield
                nc.scalar.activation(
                    out=qkt[:, jt, th * 512:(th + 1) * 512], in_=ps,
                    func=Act.Identity, bias=bqkb[:, jt:jt + 1], scale=1.0)

    def vb_gen(jh):
        """v[t, h, d] token-major for heads 8*jh..8*jh+7 (+bias via ones-row
        matmul), with a bf16 ones column at d=64 for the fused denominator."""
        if jh == 0:
            for tt in range(TT):
                nc.sync.dma_start(out=vt[:, tt, :, DH], in_=onesbf)
        wvtiles = []
        for et in range(ET):
            wt = p_wqk.tile([128, 512], F32R, tag="wqk", name="wt")
            nc.sync.dma_start(out=wt, in_=wvT[et * 128:(et + 1) * 128,
                                             jh * 512:(jh + 1) * 512])
            wvtiles.append(wt)
        for tt in range(TT):
            ps = mm_psum("mm")
            for et in range(ET):
                nc.tensor.matmul(
                    ps,
                    xt[:, et, tt * 128:(tt + 1) * 128],
                    wvtiles[et],
                    start=(et == 0), stop=False)
                yield
            nc.tensor.matmul(
                ps, onesb[0:1, 0:128], bvb[0:1, jh * 512:(jh + 1) * 512],
                start=False, stop=True)
            yield
            nc.vector.tensor_copy(
                out=vt[:, tt, jh * 8:(jh + 1) * 8, 0:DH],
                in_=ps.rearrange("p (h d) -> p h d", d=DH))

    # ---- attention generator (yields once per tk-iteration) ---------------
    LAG = 3

    def att_gen(c, a, p_esc, p_nrm):
        cs = c * 512
        last_it = 4 * c + 3
        qj = a                             # q tile of the pair
        kj = (JQK // 2) // 128 + a         # k tile of the pair
        avps = [psum.tile([128, 512], F32, tag=f"av{p}", bufs=1,
                          name=f"avp{p}") for p in range(2)]
        pend = []

        def emit_av(it, sub, clen, esc):
            for p in range(2):
                nc.tensor.matmul(
                    avps[p][0:DH + 1, sub:sub + clen],
                    vt[:, it, 2 * a + p, :],
                    esc[:, p, :clen],
                    start=(it == 0), stop=(it == last_it),
                    skip_group_check=True)

        for it in range(last_it + 1):
            n0 = it * 128
            lo = max(n0, cs)
            sub = lo - cs
            clen = 512 - sub
            scp = psum.tile([128, 2, 512], F32, tag="sc", bufs=2, name="scp")
            for p in range(2):             # paired heads: row-tiled matmuls
                pb = p * 64
                nc.tensor.matmul(
                    scp[:, p, :clen],
                    qkt[pb:pb + 64, kj, n0:n0 + 128],
                    qkt[pb:pb + 64, qj, lo:lo + clen],
                    start=True, stop=True)
            esc = p_esc.tile([128, 2, 512], BF16, tag="esc", name="esc")
            nc.scalar.activation(out=esc[:, :, :clen], in_=scp[:, :, :clen],
                                 func=Act.Exp, scale=1.0 / 8.0)
            if n0 >= cs:                   # diagonal block: causal mask,
                nc.vector.tensor_mul(      # off the PE chain thanks to LAG
                    esc[:, :, 0:128], esc[:, :, 0:128],
                    mask01b[:, None, :].broadcast_to([128, 2, 128]))
            pend.append((it, sub, clen, esc))
            if len(pend) > LAG:
                emit_av(*pend.pop(0))
            yield
        for args in pend:
            emit_av(*args)
        for p in range(2):                 # drain unnormalized y + denom row
            h = 2 * a + p
            nc.vector.tensor_copy(out=yt[p * 64:p * 64 + 64, qj,
                                         cs:cs + 512],
                                  in_=avps[p][0:DH, :])
            # engines can only address partition bases that are multiples of
            # 32, so stage the denominator row at partition 64 and DMA-
            # scatter it (partition-agnostic) into denall's row.
            stg = p_nrm.tile([128, 512], F32, tag="stg", bufs=2, name="stg")
            nc.vector.tensor_copy(out=stg[64:65, :],
                                  in_=avps[p][DH:DH + 1, :])
            nc.sync.dma_start(out=denall[32 * c + h:32 * c + h + 1, :],
                              in_=stg[64:65, :])

    def norm_unit(c, p_nrm):
        """Batched 1/denom for chunk c, bounced through DRAM and partition-
        broadcast back; normalizes y in place on DVE."""
        r0 = 32 * c
        cs = c * 512
        with nc.allow_low_precision(reason="fp32 reciprocal feeding an f32r "
                                    "multiply; well inside tolerance"):
            nc.vector.reciprocal(out=recall[r0:r0 + 16, :],
                                 in_=denall[r0:r0 + 16, :])
        nc.sync.dma_start(out=dden[r0:r0 + 16, :], in_=recall[r0:r0 + 16, :])
        for a in range(H // 2):
            rb = p_nrm.tile([128, 512], F32, tag="rb", bufs=2, name="rb")
            for p in range(2):
                row = dden[r0 + 2 * a + p:r0 + 2 * a + p + 1, :]
                src = bass.AP(tensor=row.tensor, offset=row.offset,
                              ap=[[0, 64]] + list(row.ap)[1:])
                nc.sync.dma_start(out=rb[p * 64:(p + 1) * 64, :], in_=src)
            nc.vector.tensor_mul(yt[:, a, cs:cs + 512],
                                 yt[:, a, cs:cs + 512], rb)

    # ---- drivers ----------------------------------------------------------
    def run_dense(dense, n=None):
        steps = 0
        while dense and (n is None or steps < n):
            try:
                next(dense[0])
                steps += 1
            except StopIteration:
                dense.pop(0)
        return steps

    def drive(att_units, dense, ratio=5):
        att_units = list(att_units)
        while att_units:
            try:
                next(att_units[0])
            except StopIteration:
                att_units.pop(0)
                continue
            run_dense(dense, ratio)
        run_dense(dense)

    # S1: dense deps for attention pairs 0-3
    dense1 = [qk_gen(0), qk_gen(2)] + ([vb_gen(0)] if PHASE_LIMIT >= 2 else [])
    run_dense(dense1)

    # S2: attention pairs 0-3 (both chunks) over the remaining qkv work
    p_esc1 = tc.alloc_tile_pool(name="p_esc1", bufs=4)
    p_nrm1 = tc.alloc_tile_pool(name="p_nrm1", bufs=1)
    dense2 = [qk_gen(1), qk_gen(3)] + ([vb_gen(1)] if PHASE_LIMIT >= 2 else [])
    att2 = [att_gen(c, a, p_esc1, p_nrm1)
            for a in range(4) for c in range(NT)] if PHASE_LIMIT >= 3 else []
    drive(att2, dense2)
    p_nrm1.release()
    p_esc1.release()
    p_wqk.release()
    p_x.release()

    # S3: remaining attention; out_proj weight prefetch + chunk-0 out_proj
    p_w3 = tc.alloc_tile_pool(name="p_w3", bufs=16)
    p_z = tc.alloc_tile_pool(name="p_z", bufs=1)
    p_esc2 = tc.alloc_tile_pool(name="p_esc2", bufs=4)
    p_nrm2 = tc.alloc_tile_pool(name="p_nrm2", bufs=1)
    zt = p_z.tile([128, ET, T], F32R)
    wout_tiles = []
    if PHASE_LIMIT >= 4:
        for og in range(2):
            for et in range(ET):
                wt = p_w3.tile([128, 512], F32R, tag="w3", name="wt3")
                nc.sync.dma_start(
                    out=wt, in_=woutT[et * 128:(et + 1) * 128,
                                      og * 512:(og + 1) * 512])
                wout_tiles.append(wt)

    def oproj_gen(th):
        for og in range(2):
            for os_ in range(4):
                ot = og * 4 + os_
                ps = mm_psum("mm")
                for et in range(ET):
                    nc.tensor.matmul(
                        ps,
                        wout_tiles[og * ET + et][:, os_ * 128:(os_ + 1) * 128],
                        yt[:, et, th * 512:(th + 1) * 512],
                        start=(et == 0), stop=(et == ET - 1))
                    yield
                nc.scalar.activation(
                    out=zt[:, ot, th * 512:(th + 1) * 512], in_=ps,
                    func=Act.Identity, bias=boutb[:, ot:ot + 1], scale=1.0)

    def cproj_gen(wts, og, th):
        for os_ in range(4):
            ot = og * 4 + os_
            ps = mm_psum("mm")
            for et in range(ET):
                nc.tensor.matmul(
                    ps,
                    wts[et][:, os_ * 128:(os_ + 1) * 128],
                    zt[:, et, th * 512:(th + 1) * 512],
                    start=(et == 0), stop=(et == ET - 1))
                yield
            ob = p_out.tile([128, 512], F32, tag="ob", name="ob")
            nc.scalar.activation(out=ob, in_=ps, func=Act.Identity,
                                 bias=bcb[:, ot:ot + 1], scale=1.0)
            nc.sync.dma_start(
                out=outT[ot * 128:(ot + 1) * 128, th * 512:(th + 1) * 512],
                in_=ob)

    if PHASE_LIMIT >= 3:
        drive([att_gen(0, a, p_esc2, p_nrm2) for a in range(4, 8)], [])
        norm_unit(0, p_nrm2)
        drive([att_gen(1, a, p_esc2, p_nrm2) for a in range(4, 8)],
              [oproj_gen(0)] if PHASE_LIMIT >= 4 else [])
        norm_unit(1, p_nrm2)
    p_nrm2.release()
    p_esc2.release()

    # S4: c_proj — its own weight pool so zt-chunk-0 c_proj can run ahead of
    # oproj(1) (which waits on the chunk-1 normalization chain).
    p_wc = tc.alloc_tile_pool(name="p_wc", bufs=9)
    p_out = tc.alloc_tile_pool(name="p_out", bufs=2)

    def load_wc(og):
        wts = []
        for et in range(ET):
            wt = p_wc.tile([128, 512], F32R, tag="wc", name="wtc")
            nc.sync.dma_start(out=wt, in_=wcT[et * 128:(et + 1) * 128,
                                             og * 512:(og + 1) * 512])
            wts.append(wt)
        return wts

    if PHASE_LIMIT >= 4:
        wc0 = load_wc(0)
        run_dense([cproj_gen(wc0, 0, 0)])
        run_dense([oproj_gen(1)])
        wc1 = load_wc(1)
        run_dense([cproj_gen(wc0, 0, 1)])
        run_dense([cproj_gen(wc1, 1, 0), cproj_gen(wc1, 1, 1)])
    p_out.release()
    p_wc.release()
    p_z.release()
    p_w3.release()
    p_v.release()
    p_qk.release()
    p_y.release()
    p_den.release()
    p_dram.release()
    psum.release()
    consts.release()


def _build():
    if "nc" in _CACHE:
        return _CACHE["nc"]
    nc = bacc.Bacc("TRN2", target_bir_lowering=False, debug=False,
                   enable_asserts=True, num_devices=8)
    d = nc.dram_tensor
    aps = [
        d("xT", [E, T], F32R, kind="ExternalInput").ap(),
        d("wqkT", [E, JQK], F32R, kind="ExternalInput").ap(),
        d("wvT", [E, E], F32R, kind="ExternalInput").ap(),
        d("bqk", [128, JQK // 128], F32, kind="ExternalInput").ap(),
        d("bvrow", [1, E], F32R, kind="ExternalInput").ap(),
        d("woutT", [E, E], F32R, kind="ExternalInput").ap(),
        d("bout", [128, E // 128], F32, kind="ExternalInput").ap(),
        d("wcT", [E, E], F32R, kind="ExternalInput").ap(),
        d("bc", [128, E // 128], F32, kind="ExternalInput").ap(),
        d("mask01", [128, 128], BF16, kind="ExternalInput").ap(),
        d("ones", [128, 128], F32R, kind="ExternalInput").ap(),
        d("onesbf", [128, H], BF16, kind="ExternalInput").ap(),
        d("outT", [E, T], F32, kind="ExternalOutput").ap(),
    ]
    with tile.TileContext(nc) as tc:
        _emit(nc, tc, aps)
    nc.compile()
    _CACHE["nc"] = nc
    return nc


def _host_inputs(x, in_proj_w, in_proj_b, out_proj_w, out_proj_b,
                 c_proj_w, c_proj_b):
    f = np.float32
    x = np.ascontiguousarray(np.asarray(x, f))
    in_proj_w = np.asarray(in_proj_w, f)
    in_proj_b = np.asarray(in_proj_b, f)
    shared = {
        "wqkT": np.ascontiguousarray(in_proj_w[:JQK].T),
        "wvT": np.ascontiguousarray(in_proj_w[JQK:].T),
        "bqk": np.ascontiguousarray(in_proj_b[:JQK].reshape(JQK // 128, 128).T),
        "bvrow": np.ascontiguousarray(in_proj_b[JQK:].reshape(1, E)),
        "woutT": np.ascontiguousarray(np.asarray(out_proj_w, f).T),
        "bout": np.ascontiguousarray(
            np.asarray(out_proj_b, f).reshape(E // 128, 128).T),
        "wcT": np.ascontiguousarray(np.asarray(c_proj_w, f).T),
        "bc": np.ascontiguousarray(
            np.asarray(c_proj_b, f).reshape(E // 128, 128).T),
        "mask01": np.where(np.arange(128)[None, :] >= np.arange(128)[:, None],
                           f(1.0), f(0.0)).astype(ml_dtypes.bfloat16),
        "ones": np.ones((128, 128), f),
        "onesbf": np.ones((128, H), ml_dtypes.bfloat16),
    }
    return [{**shared, "xT": np.ascontiguousarray(x[b].T)} for b in range(B)]


def kernel(x, in_proj_w, in_proj_b, out_proj_w, out_proj_b, c_proj_w,
           c_proj_b):
    nc = _build()
    in_maps = _host_inputs(x, in_proj_w, in_proj_b, out_proj_w, out_proj_b,
                           c_proj_w, c_proj_b)
    res = run_bass_kernel_spmd(nc, in_maps, core_ids=list(range(B)),
                               trace=TRACE)
    _CACHE["last_result"] = res
    out = np.stack([res.results[b]["outT"].T for b in range(B)])
    return np.ascontiguousarray(out, dtype=np.float32)



# revision 8
# speedup vs baseline: 1.1297x; 1.1297x over previous
"""Causal self-attention block (qkv proj + 16-head causal attention + out_proj
+ c_proj) on 8 trn2 NeuronCores, data-parallel over the batch (B=8: one batch
element per core).

Layout strategy (per core, batch element b):
  - Activations are kept feature-major [feature, token] on chip so every
    linear layer is a plain   out = W_T.T @ act   matmul chain with the
    (host-pre-transposed) weight as the stationary operand. No on-device
    transposes at all.
  - Attention computes transposed scores  sT[tk, tq] = k_h.T q_h  per head
    pair (row-tiled K=64 matmuls), exp with no max-subtraction (scores here
    are bounded by a few units), causal mask as a bf16 multiply on the
    diagonal blocks, and the AV product consumes sT directly with token-major
    V tiles as the stationary operand. A fused ones-row in the V operand
    (M=65) yields the softmax denominator for free. Denominator rows are
    DMA-scattered into a [16, T] tile; 1/den via the fast approx-reciprocal
    DVE op and partition-broadcast by a K=16 indicator matmul.
  - V bias and out_proj bias are folded into the c_proj bias host-side
    (exact algebra), so only the qk bias is applied on chip.
  - PSUM->SBUF drains with bias adds run on the (otherwise idle) GpSimd
    engine; the Scalar engine only runs the attention exps.
  - All big matmuls run in float32r (TF32-like, 1 PE cycle/row at >=256
    moving cols); q/k tiles are bf16 so the small diagonal score matmuls
    avoid the f32r short-stream penalty.
"""

import sys

if "/opt/trn_rl_repo" not in sys.path:
    sys.path.insert(0, "/opt/trn_rl_repo")

import ml_dtypes
import numpy as np

import concourse.bass as bass  # noqa: F401  (bass types used via tile/bacc)
import concourse.tile as tile
from concourse import bacc, mybir
from concourse.bass_utils import run_bass_kernel_spmd

B, T, E, H = 8, 1024, 1024, 16
DH = E // H          # 64
JQK = 2 * E          # q+k fused feature dim (2048)
F32 = mybir.dt.float32
F32R = mybir.dt.float32r
BF16 = mybir.dt.bfloat16
Act = mybir.ActivationFunctionType

TRACE = False        # test harness flips this for profiled runs
_CACHE = {}


def _emit(nc, tc, aps):
    (xT, wqkT, wvT, bqk, woutT, wcT, bc, mask01, ones, onesbf, ind,
     outT) = aps
    ET = E // 128     # 8  e-tiles (contraction)
    TT = T // 128     # 8  token tiles
    NT = T // 512     # 2  512-wide token column chunks

    # The kernel is emitted as one software-pipelined stream so the in-order
    # PE queue always has dense matmul work while ACT paces the attention
    # exps:
    #   S1: qk features for pairs 0-3 (j-groups 0,2) + V heads 0-7
    #   S2: attention (c0+c1, pairs 0-3) interleaved with qk j-groups 1,3 +
    #       V heads 8-15
    #   S3: attention (c0, pairs 4-7); normalize c0; attention (c1, pairs
    #       4-7) interleaved with out_proj chunk 0 + c_proj chunk 0
    #   S4: normalize c1; out_proj chunk 1; c_proj chunk 1
    consts = tc.alloc_tile_pool(name="consts", bufs=1)
    onesb = consts.tile([128, 128], F32R, tag="onesb")
    mask01b = consts.tile([128, 128], BF16, tag="mask01b")
    bqkb = consts.tile([128, JQK // 128], F32, tag="bqkb")
    bcb = consts.tile([128, E // 128], F32, tag="bcb")
    indb = consts.tile([16, (H // 2) * 128], F32R, tag="indb")
    nc.sync.dma_start(out=onesb, in_=ones)
    nc.sync.dma_start(out=mask01b, in_=mask01)
    nc.sync.dma_start(out=bqkb, in_=bqk)
    nc.sync.dma_start(out=bcb, in_=bc)
    nc.sync.dma_start(out=indb, in_=ind)

    psum = tc.alloc_tile_pool(name="psum", bufs=1, space="PSUM")
    p_den = tc.alloc_tile_pool(name="p_den", bufs=1)
    p_y = tc.alloc_tile_pool(name="p_y", bufs=1)
    p_qk = tc.alloc_tile_pool(name="p_qk", bufs=1)
    p_v = tc.alloc_tile_pool(name="p_v", bufs=1)
    p_x = tc.alloc_tile_pool(name="p_x", bufs=1)
    p_wqk = tc.alloc_tile_pool(name="p_wqk", bufs=16)
    denall = p_den.tile([16, T], F32, tag="denall")
    recall = p_den.tile([16, T], F32R, tag="recall")
    yt = p_y.tile([128, ET, T], F32R)
    qkt = p_qk.tile([128, JQK // 128, T], BF16)
    vt = p_v.tile([128, TT, H, DH + 1], BF16)
    xt = p_x.tile([128, ET, T], F32R)

    def mm_psum(tag):
        return psum.tile([128, 512], F32, tag=tag, bufs=2, name="ps_" + tag)

    # ---- dense generators: qkv projection ---------------------------------
    def qk_gen(jg):
        """qkT[j, t] = Wqk x^T + bqk for the 512-wide feature group jg."""
        wtiles = []
        for et in range(ET):
            if jg == 0:                    # interleave x loads with group 0
                nc.sync.dma_start(out=xt[:, et, :],
                                  in_=xT[et * 128:(et + 1) * 128, :])
            wt = p_wqk.tile([128, 512], F32R, tag="wqk", name="wt")
            nc.sync.dma_start(out=wt, in_=wqkT[et * 128:(et + 1) * 128,
                                              jg * 512:(jg + 1) * 512])
            wtiles.append(wt)
        for js in range(4):
            jt = jg * 4 + js
            for th in range(NT):
                ps = mm_psum("mm")
                for et in range(ET):
                    nc.tensor.matmul(
                        ps,
                        wtiles[et][:, js * 128:(js + 1) * 128],
                        xt[:, et, th * 512:(th + 1) * 512],
                        start=(et == 0), stop=(et == ET - 1))
                    yield
                nc.scalar.activation(
                    out=qkt[:, jt, th * 512:(th + 1) * 512], in_=ps,
                    func=Act.Identity, bias=bqkb[:, jt:jt + 1], scale=1.0)

    def vb_gen(jh):
        """v[t, h, d] token-major for heads 8*jh..8*jh+7 (bias folded into
        c_proj host-side), with a bf16 ones column at d=64 for the fused
        denominator."""
        if jh == 0:
            for tt in range(TT):
                nc.sync.dma_start(out=vt[:, tt, :, DH], in_=onesbf)
        wvtiles = []
        for et in range(ET):
            wt = p_wqk.tile([128, 512], F32R, tag="wqk", name="wt")
            nc.sync.dma_start(out=wt, in_=wvT[et * 128:(et + 1) * 128,
                                             jh * 512:(jh + 1) * 512])
            wvtiles.append(wt)
        for tt in range(TT):
            ps = mm_psum("mm")
            for et in range(ET):
                nc.tensor.matmul(
                    ps,
                    xt[:, et, tt * 128:(tt + 1) * 128],
                    wvtiles[et],
                    start=(et == 0), stop=(et == ET - 1))
                yield
            nc.vector.tensor_copy(
                out=vt[:, tt, jh * 8:(jh + 1) * 8, 0:DH],
                in_=ps.rearrange("p (h d) -> p h d", d=DH))

    # ---- attention generator (yields once per tk-iteration) ---------------
    LAG = 3

    def att_gen(c, a, p_esc, p_nrm):
        cs = c * 512
        last_it = 4 * c + 3
        qj = a                             # q tile of the pair
        kj = (JQK // 2) // 128 + a         # k tile of the pair
        avps = [psum.tile([128, 512], F32, tag=f"av{p}", bufs=1,
                          name=f"avp{p}") for p in range(2)]
        pend = []

        def emit_av(it, sub, clen, esc):
            for p in range(2):
                nc.tensor.matmul(
                    avps[p][0:DH + 1, sub:sub + clen],
                    vt[:, it, 2 * a + p, :],
                    esc[:, p, :clen],
                    start=(it == 0), stop=(it == last_it),
                    skip_group_check=True)

        for it in range(last_it + 1):
            n0 = it * 128
            lo = max(n0, cs)
            sub = lo - cs
            clen = 512 - sub
            scp = psum.tile([128, 2, 512], F32, tag="sc", bufs=2, name="scp")
            for p in range(2):             # paired heads: row-tiled matmuls
                pb = p * 64
                nc.tensor.matmul(
                    scp[:, p, :clen],
                    qkt[pb:pb + 64, kj, n0:n0 + 128],
                    qkt[pb:pb + 64, qj, lo:lo + clen],
                    start=True, stop=True)
            esc = p_esc.tile([128, 2, 512], BF16, tag="esc", name="esc")
            nc.scalar.activation(out=esc[:, :, :clen], in_=scp[:, :, :clen],
                                 func=Act.Exp, scale=1.0 / 8.0)
            if n0 >= cs:                   # diagonal block: causal mask on
                nc.gpsimd.tensor_mul(      # the idle gpsimd engine (esc and
                    esc[:, :, 0:128], esc[:, :, 0:128],   # mask are SBUF)
                    mask01b[:, None, :].broadcast_to([128, 2, 128]))
            pend.append((it, sub, clen, esc))
            if len(pend) > LAG:
                emit_av(*pend.pop(0))
            yield
        for args in pend:
            emit_av(*args)
        for p in range(2):                 # drain unnormalized y + denom row
            h = 2 * a + p
            nc.vector.tensor_copy(out=yt[p * 64:p * 64 + 64, qj,
                                         cs:cs + 512],
                                  in_=avps[p][0:DH, :])
            # engines can only address partition bases that are multiples of
            # 32, so stage the denominator row at partition 64 and DMA-
            # scatter it (partition-agnostic) into denall's row.
            stg = p_nrm.tile([128, 512], F32, tag="stg", bufs=2, name="stg")
            nc.vector.tensor_copy(out=stg[64:65, :],
                                  in_=avps[p][DH:DH + 1, :])
            nc.sync.dma_start(out=denall[h:h + 1, cs:cs + 512],
                              in_=stg[64:65, :])

    def norm_unit(c):
        """1/den for chunk c via the fast approx-reciprocal (values are
        positive softmax sums, well inside its range), partition-broadcast
        by K=16 indicator matmuls; normalizes y in place on DVE."""
        cs = c * 512
        # f32r tile for the matmul operand; approx-reciprocal is a raw
        # custom-DVE op on the fp32 bit layout, which f32r shares.
        from concourse.dve_ops import (
            RECIP_APPROX_FAST_CONSTS as RC,
            RECIPROCAL_APPROX_FAST,
        )
        nc.vector._custom_dve(
            RECIPROCAL_APPROX_FAST,
            out=recall[0:16, cs:cs + 512], in0=denall[0:16, cs:cs + 512],
            s0=RC["s0"], s1=RC["s1"], imm2=RC["imm2"])
        for a in range(H // 2):
            rb = psum.tile([128, 512], F32, tag="mm", bufs=2, name="rb")
            nc.tensor.matmul(
                rb, indb[:, a * 128:(a + 1) * 128],
                recall[0:16, cs:cs + 512],
                start=True, stop=True)
            nc.vector.tensor_mul(yt[:, a, cs:cs + 512],
                                 yt[:, a, cs:cs + 512], rb)

    # ---- drivers ----------------------------------------------------------
    def run_dense(dense, n=None):
        steps = 0
        while dense and (n is None or steps < n):
            try:
                next(dense[0])
                steps += 1
            except StopIteration:
                dense.pop(0)
        return steps

    def drive(att_units, dense, ratio=5):
        att_units = list(att_units)
        while att_units:
            try:
                next(att_units[0])
            except StopIteration:
                att_units.pop(0)
                continue
            run_dense(dense, ratio)
        run_dense(dense)

    # S1: dense deps for attention pairs 0-3
    dense1 = [qk_gen(0), qk_gen(2), vb_gen(0)]
    run_dense(dense1)

    # S2: attention pairs 0-3 (both chunks) over the remaining qkv work
    p_esc1 = tc.alloc_tile_pool(name="p_esc1", bufs=4)
    p_nrm1 = tc.alloc_tile_pool(name="p_nrm1", bufs=1)
    dense2 = [qk_gen(1), qk_gen(3), vb_gen(1)]
    att2 = [att_gen(c, a, p_esc1, p_nrm1)
            for a in range(4) for c in range(NT)]
    drive(att2, dense2)
    p_nrm1.release()
    p_esc1.release()
    p_wqk.release()
    p_x.release()

    # S3: remaining attention; out_proj + c_proj weight prefetch; chunk-0
    # out_proj and c_proj overlap the chunk-1 attention.
    p_w3 = tc.alloc_tile_pool(name="p_w3", bufs=16)
    p_wc = tc.alloc_tile_pool(name="p_wc", bufs=16)
    p_z = tc.alloc_tile_pool(name="p_z", bufs=1)
    p_out = tc.alloc_tile_pool(name="p_out", bufs=2)
    p_esc2 = tc.alloc_tile_pool(name="p_esc2", bufs=4)
    p_nrm2 = tc.alloc_tile_pool(name="p_nrm2", bufs=1)
    zt = p_z.tile([128, ET, T], F32R)
    wout_tiles = []
    for og in range(2):
        for et in range(ET):
            wt = p_w3.tile([128, 512], F32R, tag="w3", name="wt3")
            nc.sync.dma_start(
                out=wt, in_=woutT[et * 128:(et + 1) * 128,
                                  og * 512:(og + 1) * 512])
            wout_tiles.append(wt)
    wc_tiles = []
    for og in range(2):
        for et in range(ET):
            wt = p_wc.tile([128, 512], F32R, tag="wc", name="wtc")
            nc.sync.dma_start(out=wt, in_=wcT[et * 128:(et + 1) * 128,
                                             og * 512:(og + 1) * 512])
            wc_tiles.append(wt)

    def oproj_gen(th):
        for og in range(2):
            for os_ in range(4):
                ot = og * 4 + os_
                ps = mm_psum("mm")
                for et in range(ET):
                    nc.tensor.matmul(
                        ps,
                        wout_tiles[og * ET + et][:, os_ * 128:(os_ + 1) * 128],
                        yt[:, et, th * 512:(th + 1) * 512],
                        start=(et == 0), stop=(et == ET - 1))
                    yield
                nc.vector.tensor_copy(
                    out=zt[:, ot, th * 512:(th + 1) * 512], in_=ps)

    def cproj_gen(og, th):
        for os_ in range(4):
            ot = og * 4 + os_
            ps = mm_psum("mm")
            for et in range(ET):
                nc.tensor.matmul(
                    ps,
                    wc_tiles[og * ET + et][:, os_ * 128:(os_ + 1) * 128],
                    zt[:, et, th * 512:(th + 1) * 512],
                    start=(et == 0), stop=(et == ET - 1))
                yield
            ob = p_out.tile([128, 512], F32, tag="ob", name="ob")
            nc.vector.tensor_scalar_add(out=ob, in0=ps,
                                        scalar1=bcb[:, ot:ot + 1])
            nc.sync.dma_start(
                out=outT[ot * 128:(ot + 1) * 128, th * 512:(th + 1) * 512],
                in_=ob)

    drive([att_gen(0, a, p_esc2, p_nrm2) for a in range(4, 8)], [])
    norm_unit(0)
    drive([att_gen(1, a, p_esc2, p_nrm2) for a in range(4, 8)],
          [oproj_gen(0), cproj_gen(0, 0), cproj_gen(1, 0)])
    norm_unit(1)
    run_dense([oproj_gen(1)])
    run_dense([cproj_gen(0, 1), cproj_gen(1, 1)])
    p_nrm2.release()
    p_esc2.release()
    p_out.release()
    p_z.release()
    p_wc.release()
    p_w3.release()
    p_v.release()
    p_qk.release()
    p_y.release()
    p_den.release()
    psum.release()
    consts.release()


def _build():
    if "nc" in _CACHE:
        return _CACHE["nc"]
    nc = bacc.Bacc("TRN2", target_bir_lowering=False, debug=False,
                   enable_asserts=True, num_devices=8)
    d = nc.dram_tensor
    aps = [
        d("xT", [E, T], F32R, kind="ExternalInput").ap(),
        d("wqkT", [E, JQK], F32R, kind="ExternalInput").ap(),
        d("wvT", [E, E], F32R, kind="ExternalInput").ap(),
        d("bqk", [128, JQK // 128], F32, kind="ExternalInput").ap(),
        d("woutT", [E, E], F32R, kind="ExternalInput").ap(),
        d("wcT", [E, E], F32R, kind="ExternalInput").ap(),
        d("bc", [128, E // 128], F32, kind="ExternalInput").ap(),
        d("mask01", [128, 128], BF16, kind="ExternalInput").ap(),
        d("ones", [128, 128], F32R, kind="ExternalInput").ap(),
        d("onesbf", [128, H], BF16, kind="ExternalInput").ap(),
        d("ind", [16, (H // 2) * 128], F32R, kind="ExternalInput").ap(),
        d("outT", [E, T], F32, kind="ExternalOutput").ap(),
    ]
    with tile.TileContext(nc) as tc:
        _emit(nc, tc, aps)
    nc.compile()
    _CACHE["nc"] = nc
    return nc


def _host_inputs(x, in_proj_w, in_proj_b, out_proj_w, out_proj_b,
                 c_proj_w, c_proj_b):
    f = np.float32
    x = np.ascontiguousarray(np.asarray(x, f))
    in_proj_w = np.asarray(in_proj_w, f)
    in_proj_b = np.asarray(in_proj_b, f)
    out_proj_w = np.asarray(out_proj_w, f)
    out_proj_b = np.asarray(out_proj_b, f)
    c_proj_w = np.asarray(c_proj_w, f)
    c_proj_b = np.asarray(c_proj_b, f)
    # exact algebraic folds: v-bias and out_proj bias ride into c_proj's bias
    #   z = Wout y + (opb + Wout bv);  out = Wc z + cpb
    #   => out = Wc (Wout y) + [cpb + Wc (opb + Wout bv)]
    bout_eff = out_proj_b + out_proj_w @ in_proj_b[JQK:]
    bc_eff = c_proj_b + c_proj_w @ bout_eff
    # indicator for the denominator partition-broadcast:
    # ind[k, a*128+j] = 1 iff k == 2a + j//64
    ind = np.zeros((16, (H // 2) * 128), f)
    for a in range(H // 2):
        ind[2 * a, a * 128:a * 128 + 64] = 1.0
        ind[2 * a + 1, a * 128 + 64:(a + 1) * 128] = 1.0
    shared = {
        "wqkT": np.ascontiguousarray(in_proj_w[:JQK].T),
        "wvT": np.ascontiguousarray(in_proj_w[JQK:].T),
        "bqk": np.ascontiguousarray(in_proj_b[:JQK].reshape(JQK // 128, 128).T),
        "woutT": np.ascontiguousarray(out_proj_w.T),
        "wcT": np.ascontiguousarray(c_proj_w.T),
        "bc": np.ascontiguousarray(bc_eff.reshape(E // 128, 128).T),
        "mask01": np.where(np.arange(128)[None, :] >= np.arange(128)[:, None],
                           f(1.0), f(0.0)).astype(ml_dtypes.bfloat16),
        "ones": np.ones((128, 128), f),
        "onesbf": np.ones((128, H), ml_dtypes.bfloat16),
        "ind": ind,
    }
    return [{**shared, "xT": np.ascontiguousarray(x[b].T)} for b in range(B)]


def kernel(x, in_proj_w, in_proj_b, out_proj_w, out_proj_b, c_proj_w,
           c_proj_b):
    nc = _build()
    in_maps = _host_inputs(x, in_proj_w, in_proj_b, out_proj_w, out_proj_b,
                           c_proj_w, c_proj_b)
    res = run_bass_kernel_spmd(nc, in_maps, core_ids=list(range(B)),
                               trace=TRACE)
    _CACHE["last_result"] = res
    out = np.stack([res.results[b]["outT"].T for b in range(B)])
    return np.ascontiguousarray(out, dtype=np.float32)


# revision 16
# speedup vs baseline: 1.2363x; 1.0944x over previous
"""Causal self-attention block (qkv proj + 16-head causal attention + out_proj
+ c_proj) on 8 trn2 NeuronCores, data-parallel over the batch (B=8: one batch
element per core).

Layout strategy (per core, batch element b):
  - Activations are kept feature-major [feature, token] on chip so every
    linear layer is a plain   out = W_T.T @ act   matmul chain with the
    (host-pre-transposed) weight as the stationary operand. No on-device
    transposes at all.
  - All projection matmuls run in bf16 (1 PE cycle/row; verified 3.8e-3 rel
    error on the full pipeline vs the 2e-2 budget); PSUM accumulates fp32.
  - Attention computes transposed scores  sT[tk, tq] = k_h.T q_h  per head
    pair (row-tiled K=64 matmuls), exp with no max-subtraction (scores here
    are bounded by a few units), causal mask as a bf16 multiply on the
    diagonal blocks on the otherwise-idle gpsimd engine, and the AV product
    consumes sT directly with token-major V tiles as the stationary operand.
    A fused ones-row in the V operand (M=65) yields the softmax denominator
    for free.
  - Denominator rows are DMA-scattered into per-half [8, T] tiles; 1/den via
    the fast approx-reciprocal DVE op, partition-broadcast by K=8 indicator
    matmuls into PSUM, and normalized into y by DVE multiplies. Each
    half-chunk normalizes as soon as its 4 attention pairs finish, so only
    the (chunk1, heads 8-15) normalization sits on the critical tail.
  - V bias and out_proj bias are folded into the c_proj bias host-side
    (exact algebra), so only the qk bias is applied on chip.
  - Schedule: S1 computes qk pairs 0-3 + V heads 0-7; S2 runs attention for
    pairs 0-3 (both chunks) and pairs 4-7 (chunk 0) over the remaining
    qkv projection work; S3 runs attention pairs 4-7 (chunk 1) over
    out_proj + c_proj of chunk 0; S4 drains out_proj + c_proj of chunk 1.
"""

import sys

if "/opt/trn_rl_repo" not in sys.path:
    sys.path.insert(0, "/opt/trn_rl_repo")

import ml_dtypes
import numpy as np

import concourse.bass as bass  # noqa: F401  (bass types used via tile/bacc)
import concourse.tile as tile
from concourse import bacc, mybir
from concourse.bass_utils import run_bass_kernel_spmd
from concourse.dve_ops import (
    RECIP_APPROX_FAST_CONSTS as RC,
    RECIPROCAL_APPROX_FAST,
)

B, T, E, H = 8, 1024, 1024, 16
DH = E // H          # 64
JQK = 2 * E          # q+k fused feature dim (2048)
F32 = mybir.dt.float32
F32R = mybir.dt.float32r
BF16 = mybir.dt.bfloat16
Act = mybir.ActivationFunctionType

TRACE = False        # test harness flips this for profiled runs
_CACHE = {}


def _emit(nc, tc, aps):
    (xT, wqkT, wvT, bqk, woutT, wcT, bc, mask01, onesbf, ind, outT) = aps
    ET = E // 128     # 8  e-tiles (contraction)
    TT = T // 128     # 8  token tiles
    NT = T // 512     # 2  512-wide token column chunks

    consts = tc.alloc_tile_pool(name="consts", bufs=1)
    mask01b = consts.tile([128, 128], BF16, tag="mask01b")
    bqkb = consts.tile([128, JQK // 128], F32, tag="bqkb")
    bcb = consts.tile([128, E // 128], F32, tag="bcb")
    indb = consts.tile([8, (H // 4) * 128], F32R, tag="indb")

    psum = tc.alloc_tile_pool(name="psum", bufs=1, space="PSUM")
    p_den = tc.alloc_tile_pool(name="p_den", bufs=1)
    p_y = tc.alloc_tile_pool(name="p_y", bufs=1)
    p_qk = tc.alloc_tile_pool(name="p_qk", bufs=1)
    p_v = tc.alloc_tile_pool(name="p_v", bufs=1)
    p_x = tc.alloc_tile_pool(name="p_x", bufs=1)
    p_wqk = tc.alloc_tile_pool(name="p_wqk", bufs=16)
    den = [p_den.tile([8, T], F32, tag=f"den{i}", name=f"den{i}")
           for i in range(2)]
    rec = [p_den.tile([8, T], F32R, tag=f"rec{i}", name=f"rec{i}")
           for i in range(2)]
    yt = p_y.tile([128, ET, T], BF16)
    qkt = p_qk.tile([128, JQK // 128, T], BF16)
    vt = p_v.tile([128, TT, H, DH + 1], BF16)
    xt = p_x.tile([128, ET, T], BF16)

    def mm_psum(tag):
        return psum.tile([128, 512], F32, tag=tag, bufs=2, name="ps_" + tag)

    # ---- dense generators: qkv projection ---------------------------------
    def qk_gen(jg, first=False):
        """qkT[j, t] = Wqk x^T + bqk for the 512-wide feature group jg."""
        wtiles = []
        for et in range(ET):
            if first:                      # interleave x loads with group 0
                nc.sync.dma_start(out=xt[:, et, :],
                                  in_=xT[et * 128:(et + 1) * 128, :])
            wt = p_wqk.tile([128, 512], BF16, tag="wqk", name="wt")
            nc.sync.dma_start(out=wt, in_=wqkT[et * 128:(et + 1) * 128,
                                              jg * 512:(jg + 1) * 512])
            wtiles.append(wt)
        if first:                          # consts ride behind the x tiles
            nc.sync.dma_start(out=bqkb, in_=bqk)
            nc.sync.dma_start(out=mask01b, in_=mask01)
            for tt in range(TT):
                nc.sync.dma_start(out=vt[:, tt, :, DH], in_=onesbf)
        for js in range(4):
            jt = jg * 4 + js
            for th in range(NT):
                ps = mm_psum("mm")
                for et in range(ET):
                    nc.tensor.matmul(
                        ps,
                        wtiles[et][:, js * 128:(js + 1) * 128],
                        xt[:, et, th * 512:(th + 1) * 512],
                        start=(et == 0), stop=(et == ET - 1))
                    yield
                nc.scalar.activation(
                    out=qkt[:, jt, th * 512:(th + 1) * 512], in_=ps,
                    func=Act.Identity, bias=bqkb[:, jt:jt + 1], scale=1.0)

    def vb_gen(jh):
        """v[t, h, d] token-major for heads 8*jh..8*jh+7 (bias folded into
        c_proj host-side), with a bf16 ones column at d=64 for the fused
        denominator."""
        wvtiles = []
        for et in range(ET):
            wt = p_wqk.tile([128, 512], BF16, tag="wqk", name="wt")
            nc.sync.dma_start(out=wt, in_=wvT[et * 128:(et + 1) * 128,
                                             jh * 512:(jh + 1) * 512])
            wvtiles.append(wt)
        for tt in range(TT):
            ps = mm_psum("mm")
            for et in range(ET):
                nc.tensor.matmul(
                    ps,
                    xt[:, et, tt * 128:(tt + 1) * 128],
                    wvtiles[et],
                    start=(et == 0), stop=(et == ET - 1))
                yield
            nc.vector.tensor_copy(
                out=vt[:, tt, jh * 8:(jh + 1) * 8, 0:DH],
                in_=ps.rearrange("p (h d) -> p h d", d=DH))

    # ---- attention generator (yields once per tk-iteration) ---------------
    LAG = 3

    def att_gen(c, a, p_esc, p_nrm):
        cs = c * 512
        last_it = 4 * c + 3
        qj = a                             # q tile of the pair
        kj = (JQK // 2) // 128 + a         # k tile of the pair
        avps = [psum.tile([128, 512], F32, tag=f"av{p}", bufs=1,
                          name=f"avp{p}") for p in range(2)]
        pend = []

        def emit_av(it, sub, clen, esc):
            for p in range(2):
                nc.tensor.matmul(
                    avps[p][0:DH + 1, sub:sub + clen],
                    vt[:, it, 2 * a + p, :],
                    esc[:, p, :clen],
                    start=(it == 0), stop=(it == last_it),
                    skip_group_check=True)

        for it in range(last_it + 1):
            n0 = it * 128
            lo = max(n0, cs)
            sub = lo - cs
            clen = 512 - sub
            scp = psum.tile([128, 2, 512], F32, tag="sc", bufs=2, name="scp")
            for p in range(2):             # paired heads: row-tiled matmuls
                pb = p * 64
                nc.tensor.matmul(
                    scp[:, p, :clen],
                    qkt[pb:pb + 64, kj, n0:n0 + 128],
                    qkt[pb:pb + 64, qj, lo:lo + clen],
                    start=True, stop=True)
            esc = p_esc.tile([128, 2, 512], BF16, tag="esc", name="esc")
            nc.scalar.activation(out=esc[:, :, :clen], in_=scp[:, :, :clen],
                                 func=Act.Exp, scale=1.0 / 8.0)
            if n0 >= cs:                   # diagonal block: causal mask on
                nc.gpsimd.tensor_mul(      # the idle gpsimd engine (esc and
                    esc[:, :, 0:128], esc[:, :, 0:128],   # mask are SBUF)
                    mask01b[:, None, :].broadcast_to([128, 2, 128]))
            pend.append((it, sub, clen, esc))
            if len(pend) > LAG:
                emit_av(*pend.pop(0))
            yield
        for args in pend:
            emit_av(*args)
        for p in range(2):                 # drain unnormalized y + denom row
            h = 2 * a + p
            nc.vector.tensor_copy(out=yt[p * 64:p * 64 + 64, qj,
                                         cs:cs + 512],
                                  in_=avps[p][0:DH, :])
            # engines can only address partition bases that are multiples of
            # 32, so stage the denominator row at partition 64 and DMA-
            # scatter it (partition-agnostic) into the den tile's row.
            stg = p_nrm.tile([128, 512], F32, tag="stg", bufs=2, name="stg")
            nc.vector.tensor_copy(out=stg[64:65, :],
                                  in_=avps[p][DH:DH + 1, :])
            nc.sync.dma_start(out=den[a // 4][h % 8:h % 8 + 1, cs:cs + 512],
                              in_=stg[64:65, :])

    def norm_half(c, half, rb_tags=None):
        """1/den for (chunk c, pair-half) via the fast approx-reciprocal
        (softmax sums are positive, well in range; raw custom-DVE op on the
        fp32 bit layout, which f32r shares), partition-broadcast by K=8
        indicator matmuls; normalizes y in place on DVE."""
        cs = c * 512
        nc.vector._custom_dve(
            RECIPROCAL_APPROX_FAST,
            out=rec[half][0:8, cs:cs + 512], in0=den[half][0:8, cs:cs + 512],
            s0=RC["s0"], s1=RC["s1"], imm2=RC["imm2"])
        for ap in range(4):
            a = half * 4 + ap
            tag, bufs = (rb_tags[ap] if rb_tags else ("mm", 2))
            rb = psum.tile([128, 512], F32, tag=tag, bufs=bufs, name="rb")
            nc.tensor.matmul(
                rb, indb[:, ap * 128:(ap + 1) * 128],
                rec[half][0:8, cs:cs + 512],
                start=True, stop=True)
            nc.vector.tensor_mul(yt[:, a, cs:cs + 512],
                                 yt[:, a, cs:cs + 512], rb)

    # ---- drivers ----------------------------------------------------------
    def run_dense(dense, n=None):
        steps = 0
        while dense and (n is None or steps < n):
            try:
                next(dense[0])
                steps += 1
            except StopIteration:
                dense.pop(0)
        return steps

    def drive(att_units, dense, callbacks=None, ratio=5):
        """Round-robin one att unit at a time against the dense stream.
        callbacks[i] (if set) runs right after att unit i completes."""
        att_units = list(att_units)
        callbacks = callbacks or {}
        i = 0
        while att_units:
            try:
                next(att_units[0])
            except StopIteration:
                att_units.pop(0)
                cb = callbacks.pop(i, None)
                if cb:
                    cb()
                i += 1
                continue
            run_dense(dense, ratio)
        run_dense(dense)

    # S1: dense deps for attention pairs 0-3
    run_dense([qk_gen(0, first=True), qk_gen(2), vb_gen(0)])

    # S2: attention pairs 0-3 (both chunks) + pairs 4-7 (chunk 0) over the
    # remaining qkv work; half-chunk norms fire as their pairs complete.
    p_esc1 = tc.alloc_tile_pool(name="p_esc1", bufs=4)
    p_nrm1 = tc.alloc_tile_pool(name="p_nrm1", bufs=1)
    nc.sync.dma_start(out=indb, in_=ind)
    nc.sync.dma_start(out=bcb, in_=bc)
    dense2 = [qk_gen(1), qk_gen(3), vb_gen(1)]
    att2 = ([att_gen(c, a, p_esc1, p_nrm1)
             for a in range(4) for c in range(NT)]
            + [att_gen(0, a, p_esc1, p_nrm1) for a in range(4, 8)])
    cbs = {
        8: lambda: norm_half(0, 0),        # after att(c0, pair 4): pairs 0-3
        9: lambda: norm_half(1, 0),        # of both chunks long complete, so
    }                                      # the recips never stall the PE
    drive(att2, dense2, cbs)
    p_nrm1.release()
    p_esc1.release()
    p_wqk.release()
    p_x.release()

    # S3: attention pairs 4-7 (chunk 1) over out_proj + c_proj of chunk 0
    p_w3 = tc.alloc_tile_pool(name="p_w3", bufs=16)
    p_wc = tc.alloc_tile_pool(name="p_wc", bufs=16)
    p_z = tc.alloc_tile_pool(name="p_z", bufs=1)
    p_out = tc.alloc_tile_pool(name="p_out", bufs=2)
    p_esc2 = tc.alloc_tile_pool(name="p_esc2", bufs=4)
    p_nrm2 = tc.alloc_tile_pool(name="p_nrm2", bufs=1)
    zt = p_z.tile([128, ET, T], BF16)
    wout_tiles = []
    for og in range(2):
        for et in range(ET):
            wt = p_w3.tile([128, 512], BF16, tag="w3", name="wt3")
            nc.sync.dma_start(
                out=wt, in_=woutT[et * 128:(et + 1) * 128,
                                  og * 512:(og + 1) * 512])
            wout_tiles.append(wt)
    wc_tiles = []
    for og in range(2):
        for et in range(ET):
            wt = p_wc.tile([128, 512], BF16, tag="wc", name="wtc")
            nc.sync.dma_start(out=wt, in_=wcT[et * 128:(et + 1) * 128,
                                             og * 512:(og + 1) * 512])
            wc_tiles.append(wt)

    def oproj_gen(th):
        for og in range(2):
            for os_ in range(4):
                ot = og * 4 + os_
                ps = mm_psum("mm")
                for et in range(ET):
                    nc.tensor.matmul(
                        ps,
                        wout_tiles[og * ET + et][:, os_ * 128:(os_ + 1) * 128],
                        yt[:, et, th * 512:(th + 1) * 512],
                        start=(et == 0), stop=(et == ET - 1))
                    yield
                nc.vector.tensor_copy(
                    out=zt[:, ot, th * 512:(th + 1) * 512], in_=ps)

    def cproj_gen(og, th):
        for os_ in range(4):
            ot = og * 4 + os_
            ps = mm_psum("mm")
            for et in range(ET):
                nc.tensor.matmul(
                    ps,
                    wc_tiles[og * ET + et][:, os_ * 128:(os_ + 1) * 128],
                    zt[:, et, th * 512:(th + 1) * 512],
                    start=(et == 0), stop=(et == ET - 1))
                yield
            ob = p_out.tile([128, 512], F32, tag="ob", name="ob")
            nc.vector.tensor_scalar_add(out=ob, in0=ps,
                                        scalar1=bcb[:, ot:ot + 1])
            nc.sync.dma_start(
                out=outT[ot * 128:(ot + 1) * 128, th * 512:(th + 1) * 512],
                in_=ob)

    att3 = [att_gen(1, a, p_esc2, p_nrm2) for a in range(4, 8)]
    dense3 = []

    def open_dense3():
        # (c0, heads 8-15) den rows landed at the end of S2; normalizing here
        # keeps the recip chain off the in-order PE queue's critical path,
        # and out_proj/c_proj chunk 0 only emit after their y is final.
        norm_half(0, 1)
        dense3.extend([oproj_gen(0), cproj_gen(0, 0), cproj_gen(1, 0)])

    drive(att3, dense3, {0: open_dense3})

    # S4: final half-norm on 4 distinct PSUM slots (attention is done, so
    # the av slots are free — no write-after-read serialization), then the
    # chunk-1 projections.
    norm_half(1, 1, rb_tags=(("av0", 1), ("av1", 1), ("mm", 2), ("mm", 2)))
    run_dense([oproj_gen(1)])
    run_dense([cproj_gen(0, 1), cproj_gen(1, 1)])
    p_nrm2.release()
    p_esc2.release()
    p_out.release()
    p_z.release()
    p_wc.release()
    p_w3.release()
    p_v.release()
    p_qk.release()
    p_y.release()
    p_den.release()
    psum.release()
    consts.release()


def _build():
    if "nc" in _CACHE:
        return _CACHE["nc"]
    nc = bacc.Bacc("TRN2", target_bir_lowering=False, debug=False,
                   enable_asserts=True, num_devices=8)
    d = nc.dram_tensor
    aps = [
        d("xT", [E, T], BF16, kind="ExternalInput").ap(),
        d("wqkT", [E, JQK], BF16, kind="ExternalInput").ap(),
        d("wvT", [E, E], BF16, kind="ExternalInput").ap(),
        d("bqk", [128, JQK // 128], F32, kind="ExternalInput").ap(),
        d("woutT", [E, E], BF16, kind="ExternalInput").ap(),
        d("wcT", [E, E], BF16, kind="ExternalInput").ap(),
        d("bc", [128, E // 128], F32, kind="ExternalInput").ap(),
        d("mask01", [128, 128], BF16, kind="ExternalInput").ap(),
        d("onesbf", [128, H], BF16, kind="ExternalInput").ap(),
        d("ind", [8, (H // 4) * 128], F32R, kind="ExternalInput").ap(),
        d("outT", [E, T], F32, kind="ExternalOutput").ap(),
    ]
    with tile.TileContext(nc) as tc:
        _emit(nc, tc, aps)
    nc.compile()
    _CACHE["nc"] = nc
    return nc


def _host_inputs(x, in_proj_w, in_proj_b, out_proj_w, out_proj_b,
                 c_proj_w, c_proj_b):
    f = np.float32
    bf = ml_dtypes.bfloat16
    x = np.asarray(x, f)
    in_proj_w = np.asarray(in_proj_w, f)
    in_proj_b = np.asarray(in_proj_b, f)
    out_proj_w = np.asarray(out_proj_w, f)
    out_proj_b = np.asarray(out_proj_b, f)
    c_proj_w = np.asarray(c_proj_w, f)
    c_proj_b = np.asarray(c_proj_b, f)
    # exact algebraic folds: v-bias and out_proj bias ride into c_proj's bias
    #   z = Wout y + (opb + Wout bv);  out = Wc z + cpb
    #   => out = Wc (Wout y) + [cpb + Wc (opb + Wout bv)]
    bout_eff = out_proj_b + out_proj_w @ in_proj_b[JQK:]
    bc_eff = c_proj_b + c_proj_w @ bout_eff
    # indicator for the denominator partition-broadcast:
    # ind[k, ap*128+j] = 1 iff k == 2*ap + j//64   (per half of 8 heads)
    ind = np.zeros((8, (H // 4) * 128), f)
    for ap in range(H // 4):
        ind[2 * ap, ap * 128:ap * 128 + 64] = 1.0
        ind[2 * ap + 1, ap * 128 + 64:(ap + 1) * 128] = 1.0
    shared = {
        "wqkT": np.ascontiguousarray(in_proj_w[:JQK].T).astype(bf),
        "wvT": np.ascontiguousarray(in_proj_w[JQK:].T).astype(bf),
        "bqk": np.ascontiguousarray(in_proj_b[:JQK].reshape(JQK // 128, 128).T),
        "woutT": np.ascontiguousarray(out_proj_w.T).astype(bf),
        "wcT": np.ascontiguousarray(c_proj_w.T).astype(bf),
        "bc": np.ascontiguousarray(bc_eff.reshape(E // 128, 128).T),
        "mask01": np.where(np.arange(128)[None, :] >= np.arange(128)[:, None],
                           f(1.0), f(0.0)).astype(bf),
        "onesbf": np.ones((128, H), bf),
        "ind": ind,
    }
    return [{**shared, "xT": np.ascontiguousarray(x[b].T).astype(bf)}
            for b in range(B)]


def kernel(x, in_proj_w, in_proj_b, out_proj_w, out_proj_b, c_proj_w,
           c_proj_b):
    nc = _build()
    in_maps = _host_inputs(x, in_proj_w, in_proj_b, out_proj_w, out_proj_b,
                           c_proj_w, c_proj_b)
    res = run_bass_kernel_spmd(nc, in_maps, core_ids=list(range(B)),
                               trace=TRACE)
    _CACHE["last_result"] = res
    out = np.stack([res.results[b]["outT"].T for b in range(B)])
    return np.ascontiguousarray(out, dtype=np.float32)


# revision 26
# speedup vs baseline: 1.2426x; 1.0051x over previous
"""Causal self-attention block (qkv proj + 16-head causal attention + out_proj
+ c_proj) on 8 trn2 NeuronCores, data-parallel over the batch (B=8: one batch
element per core).

Layout strategy (per core, batch element b):
  - Activations are kept feature-major [feature, token] on chip so every
    linear layer is a plain   out = W_T.T @ act   matmul chain with the
    (host-pre-transposed) weight as the stationary operand. No on-device
    transposes at all.
  - All projection matmuls run in bf16 (1 PE cycle/row; verified 3.8e-3 rel
    error on the full pipeline vs the 2e-2 budget); PSUM accumulates fp32.
  - Attention computes transposed scores  sT[tk, tq] = k_h.T q_h  per head
    pair (row-tiled K=64 matmuls), exp with no max-subtraction (scores here
    are bounded by a few units), causal mask as a bf16 multiply on the
    diagonal blocks on the otherwise-idle gpsimd engine, and the AV product
    consumes sT directly with token-major V tiles as the stationary operand.
    A fused ones-row in the V operand (M=65) yields the softmax denominator
    for free.
  - Denominator rows are DMA-scattered into per-half [8, T] tiles; 1/den via
    the fast approx-reciprocal DVE op, partition-broadcast by K=8 indicator
    matmuls into PSUM, and normalized into y by DVE multiplies. Each
    half-chunk normalizes as soon as its 4 attention pairs finish, so only
    the (chunk1, heads 8-15) normalization sits on the critical tail.
  - V bias and out_proj bias are folded into the c_proj bias host-side
    (exact algebra), so only the qk bias is applied on chip.
  - Schedule: S1 computes qk pairs 0-3 + V heads 0-7; S2 runs attention for
    pairs 0-3 (both chunks) and pairs 4-7 (chunk 0) over the remaining
    qkv projection work; S3 runs attention pairs 4-7 (chunk 1) over
    out_proj + c_proj of chunk 0; S4 drains out_proj + c_proj of chunk 1.
"""

import sys

if "/opt/trn_rl_repo" not in sys.path:
    sys.path.insert(0, "/opt/trn_rl_repo")

import ml_dtypes
import numpy as np

import concourse.bass as bass  # noqa: F401  (bass types used via tile/bacc)
import concourse.tile as tile
from concourse import bacc, mybir
from concourse.bass_utils import run_bass_kernel_spmd
from concourse.dve_ops import (
    RECIP_APPROX_FAST_CONSTS as RC,
    RECIPROCAL_APPROX_FAST,
)

B, T, E, H = 8, 1024, 1024, 16
DH = E // H          # 64
JQK = 2 * E          # q+k fused feature dim (2048)
F32 = mybir.dt.float32
F32R = mybir.dt.float32r
BF16 = mybir.dt.bfloat16
Act = mybir.ActivationFunctionType

TRACE = False        # test harness flips this for profiled runs
_CACHE = {}


def _emit(nc, tc, aps):
    (xT, wqkT, wvT, bqk, woutT, wcT, bc, mask01, ind, onesbf, outT) = aps
    ET = E // 128     # 8  e-tiles (contraction)
    TT = T // 128     # 8  token tiles
    NT = T // 512     # 2  512-wide token column chunks

    consts = tc.alloc_tile_pool(name="consts", bufs=1)
    mask01b = consts.tile([128, 128], BF16, tag="mask01b")
    bqkb = consts.tile([128, JQK // 128], F32, tag="bqkb")
    bcb = consts.tile([128, E // 128], F32, tag="bcb")
    indb = consts.tile([8, (H // 4) * 128], F32R, tag="indb")

    psum = tc.alloc_tile_pool(name="psum", bufs=1, space="PSUM")
    p_den = tc.alloc_tile_pool(name="p_den", bufs=1)
    p_y = tc.alloc_tile_pool(name="p_y", bufs=1)
    p_qk = tc.alloc_tile_pool(name="p_qk", bufs=1)
    p_v = tc.alloc_tile_pool(name="p_v", bufs=1)
    p_x = tc.alloc_tile_pool(name="p_x", bufs=1)
    p_wqk = tc.alloc_tile_pool(name="p_wqk", bufs=16)
    den = [p_den.tile([8, T], F32, tag=f"den{i}", name=f"den{i}")
           for i in range(2)]
    rec = [p_den.tile([8, T], F32R, tag=f"rec{i}", name=f"rec{i}")
           for i in range(2)]
    yt = p_y.tile([128, ET, T], BF16)
    qkt = p_qk.tile([128, JQK // 128, T], BF16)
    vt = p_v.tile([128, TT, H, DH + 1], BF16)
    xt = p_x.tile([128, ET, T], BF16)

    nc.sync.dma_start(out=indb, in_=ind)

    def mm_psum(tag):
        return psum.tile([128, 512], F32, tag=tag, bufs=2, name="ps_" + tag)

    # ---- dense generators: qkv projection ---------------------------------
    def qk_gen(jg, first=False):
        """qkT[j, t] = Wqk x^T + bqk for the 512-wide feature group jg."""
        wtiles = []
        for et in range(ET):
            if first:                      # interleave x loads with group 0
                nc.sync.dma_start(out=xt[:, et, :],
                                  in_=xT[et * 128:(et + 1) * 128, :])
            wt = p_wqk.tile([128, 512], BF16, tag="wqk", name="wt")
            nc.sync.dma_start(out=wt, in_=wqkT[et * 128:(et + 1) * 128,
                                              jg * 512:(jg + 1) * 512])
            wtiles.append(wt)
        if first:
            nc.sync.dma_start(out=bqkb, in_=bqk)
            nc.sync.dma_start(out=mask01b, in_=mask01)
            for tt in range(TT):             # fused-denominator ones column
                nc.sync.dma_start(out=vt[:, tt, :, DH], in_=onesbf)
        for js in range(4):
            jt = jg * 4 + js
            for th in range(NT):
                ps = mm_psum("mm")
                for et in range(ET):
                    nc.tensor.matmul(
                        ps,
                        wtiles[et][:, js * 128:(js + 1) * 128],
                        xt[:, et, th * 512:(th + 1) * 512],
                        start=(et == 0), stop=(et == ET - 1))
                    yield
                nc.scalar.activation(
                    out=qkt[:, jt, th * 512:(th + 1) * 512], in_=ps,
                    func=Act.Identity, bias=bqkb[:, jt:jt + 1], scale=1.0)

    def vb_gen(jh):
        """v[t, h, d] token-major for heads 8*jh..8*jh+7 (bias folded into
        c_proj host-side); weight loads ride the gpsimd queue, off the
        x/wqk critical path."""
        wvtiles = []
        for et in range(ET):
            wt = p_wqk.tile([128, 512], BF16, tag="wqk", name="wt")
            nc.sync.dma_start(out=wt, in_=wvT[et * 128:(et + 1) * 128,
                                             jh * 512:(jh + 1) * 512])
            wvtiles.append(wt)
        for tt in range(TT):
            ps = mm_psum("mm")
            for et in range(ET):
                nc.tensor.matmul(
                    ps,
                    xt[:, et, tt * 128:(tt + 1) * 128],
                    wvtiles[et],
                    start=(et == 0), stop=(et == ET - 1))
                yield
            nc.vector.tensor_copy(
                out=vt[:, tt, jh * 8:(jh + 1) * 8, 0:DH],
                in_=ps.rearrange("p (h d) -> p h d", d=DH))

    # ---- attention (yields once per tk-iteration) --------------------------
    LAG = 3

    def norm_half(c, half, rb_tags=None):
        """1/den for (chunk c, pair-half) via the fast approx-reciprocal
        (softmax sums are positive, well in range; raw custom-DVE op on the
        fp32 bit layout, which f32r shares), partition-broadcast by K=8
        indicator matmuls; normalizes y in place on DVE."""
        cs = c * 512
        nc.vector._custom_dve(
            RECIPROCAL_APPROX_FAST,
            out=rec[half][0:8, cs:cs + 512], in0=den[half][0:8, cs:cs + 512],
            s0=RC["s0"], s1=RC["s1"], imm2=RC["imm2"])
        for ap in range(4):
            a = half * 4 + ap
            tag, bufs = (rb_tags[ap] if rb_tags else ("mm", 2))
            rb = psum.tile([128, 512], F32, tag=tag, bufs=bufs, name="rb")
            nc.tensor.matmul(
                rb, indb[:, ap * 128:(ap + 1) * 128],
                rec[half][0:8, cs:cs + 512],
                start=True, stop=True)
            nc.vector.tensor_mul(yt[:, a, cs:cs + 512],
                                 yt[:, a, cs:cs + 512], rb)

    def att_gen(c, a, p_esc, p_nrm):
        cs = c * 512
        last_it = 4 * c + 3
        qj = a                             # q tile of the pair
        kj = (JQK // 2) // 128 + a         # k tile of the pair
        avps = [psum.tile([128, 512], F32, tag=f"av{p}", bufs=1,
                          name=f"avp{p}") for p in range(2)]
        pend = []

        def emit_av(it, sub, clen, esc):
            for p in range(2):
                nc.tensor.matmul(
                    avps[p][0:DH + 1, sub:sub + clen],
                    vt[:, it, 2 * a + p, :],
                    esc[:, p, :clen],
                    start=(it == 0), stop=(it == last_it),
                    skip_group_check=True)

        for it in range(last_it + 1):
            n0 = it * 128
            lo = max(n0, cs)
            sub = lo - cs
            clen = 512 - sub
            scp = psum.tile([128, 2, 512], F32, tag="sc", bufs=2, name="scp")
            for p in range(2):             # paired heads: row-tiled matmuls
                pb = p * 64
                nc.tensor.matmul(
                    scp[:, p, :clen],
                    qkt[pb:pb + 64, kj, n0:n0 + 128],
                    qkt[pb:pb + 64, qj, lo:lo + clen],
                    start=True, stop=True)
            esc = p_esc.tile([128, 2, 512], BF16, tag="esc", name="esc")
            nc.scalar.activation(out=esc[:, :, :clen], in_=scp[:, :, :clen],
                                 func=Act.Exp, scale=1.0 / 8.0)
            if n0 >= cs:                   # diagonal block: causal mask on
                nc.gpsimd.tensor_mul(      # the idle gpsimd engine (esc and
                    esc[:, :, 0:128], esc[:, :, 0:128],   # mask are SBUF)
                    mask01b[:, None, :].broadcast_to([128, 2, 128]))
            pend.append((it, sub, clen, esc))
            if len(pend) > LAG:
                emit_av(*pend.pop(0))
            yield
        for args in pend:
            emit_av(*args)
        for p in range(2):                 # drain unnormalized y + denom row
            h = 2 * a + p
            nc.vector.tensor_copy(out=yt[p * 64:p * 64 + 64, qj,
                                         cs:cs + 512],
                                  in_=avps[p][0:DH, :])
            # engines can only address partition bases that are multiples of
            # 32, so stage the denominator row at partition 64 and DMA-
            # scatter it (partition-agnostic, on the gpsimd queue) into the
            # den tile's row.
            stg = p_nrm.tile([128, 512], F32, tag="stg", bufs=2, name="stg")
            nc.vector.tensor_copy(out=stg[64:65, :],
                                  in_=avps[p][DH:DH + 1, :])
            nc.sync.dma_start(out=den[a // 4][h % 8:h % 8 + 1,
                                              cs:cs + 512],
                              in_=stg[64:65, :])

    # ---- drivers ----------------------------------------------------------
    def run_dense(dense, n=None):
        steps = 0
        while dense and (n is None or steps < n):
            try:
                next(dense[0])
                steps += 1
            except StopIteration:
                dense.pop(0)
        return steps

    def drive(att_units, dense, callbacks=None, ratio=5):
        """Round-robin one att unit at a time against the dense stream.
        callbacks[i] (if set) runs right after att unit i completes."""
        att_units = list(att_units)
        callbacks = callbacks or {}
        i = 0
        while att_units:
            try:
                next(att_units[0])
            except StopIteration:
                att_units.pop(0)
                cb = callbacks.pop(i, None)
                if cb:
                    cb()
                i += 1
                continue
            run_dense(dense, ratio)
        run_dense(dense)

    # S1: dense deps for attention pairs 0-3
    run_dense([qk_gen(0, first=True), qk_gen(2), vb_gen(0)])

    # S2: attention pairs 0-3 (both chunks) + pairs 4-7 (chunk 0) over the
    # remaining qkv work; pair-norms trail one unit behind.
    p_esc1 = tc.alloc_tile_pool(name="p_esc1", bufs=4)
    p_nrm1 = tc.alloc_tile_pool(name="p_nrm1", bufs=1)
    nc.sync.dma_start(out=bcb, in_=bc)
    dense2 = [qk_gen(1), qk_gen(3), vb_gen(1)]
    att2 = ([att_gen(c, a, p_esc1, p_nrm1)
             for a in range(4) for c in range(NT)]
            + [att_gen(0, a, p_esc1, p_nrm1) for a in range(4, 8)])
    cbs = {
        8: lambda: norm_half(0, 0),        # after att(c0, pair 4): pairs 0-3
        9: lambda: norm_half(1, 0),        # of both chunks long complete, so
    }                                      # the recips never stall the PE
    drive(att2, dense2, cbs)
    p_nrm1.release()
    p_esc1.release()
    p_wqk.release()
    p_x.release()

    # S3: attention pairs 4-7 (chunk 1) over out_proj + c_proj of chunk 0
    p_w3 = tc.alloc_tile_pool(name="p_w3", bufs=16)
    p_wc = tc.alloc_tile_pool(name="p_wc", bufs=16)
    p_z = tc.alloc_tile_pool(name="p_z", bufs=1)
    p_out = tc.alloc_tile_pool(name="p_out", bufs=2)
    p_esc2 = tc.alloc_tile_pool(name="p_esc2", bufs=4)
    p_nrm2 = tc.alloc_tile_pool(name="p_nrm2", bufs=1)
    zt = p_z.tile([128, ET, T], BF16)
    wout_tiles = []
    for og in range(2):
        for et in range(ET):
            wt = p_w3.tile([128, 512], BF16, tag="w3", name="wt3")
            nc.sync.dma_start(
                out=wt, in_=woutT[et * 128:(et + 1) * 128,
                                  og * 512:(og + 1) * 512])
            wout_tiles.append(wt)
    wc_tiles = []
    for og in range(2):
        for et in range(ET):
            wt = p_wc.tile([128, 512], BF16, tag="wc", name="wtc")
            nc.sync.dma_start(out=wt, in_=wcT[et * 128:(et + 1) * 128,
                                             og * 512:(og + 1) * 512])
            wc_tiles.append(wt)

    def oproj_gen(th):
        for og in range(2):
            for os_ in range(4):
                ot = og * 4 + os_
                ps = mm_psum("mm")
                for et in range(ET):
                    nc.tensor.matmul(
                        ps,
                        wout_tiles[og * ET + et][:, os_ * 128:(os_ + 1) * 128],
                        yt[:, et, th * 512:(th + 1) * 512],
                        start=(et == 0), stop=(et == ET - 1))
                    yield
                nc.vector.tensor_copy(
                    out=zt[:, ot, th * 512:(th + 1) * 512], in_=ps)

    def cproj_gen(og, th):
        for os_ in range(4):
            ot = og * 4 + os_
            ps = mm_psum("mm")
            for et in range(ET):
                nc.tensor.matmul(
                    ps,
                    wc_tiles[og * ET + et][:, os_ * 128:(os_ + 1) * 128],
                    zt[:, et, th * 512:(th + 1) * 512],
                    start=(et == 0), stop=(et == ET - 1))
                yield
            ob = p_out.tile([128, 512], F32, tag="ob", name="ob")
            nc.vector.tensor_scalar_add(out=ob, in0=ps,
                                        scalar1=bcb[:, ot:ot + 1])
            nc.sync.dma_start(
                out=outT[ot * 128:(ot + 1) * 128, th * 512:(th + 1) * 512],
                in_=ob)

    att3 = [att_gen(1, a, p_esc2, p_nrm2) for a in range(4, 8)]
    dense3 = []

    def open_dense3():
        # (c0, heads 8-15) den rows landed at the end of S2; normalizing here
        # keeps the recip chain off the in-order PE queue's critical path,
        # and out_proj/c_proj chunk 0 only emit after their y is final.
        norm_half(0, 1)
        dense3.extend([oproj_gen(0), cproj_gen(0, 0), cproj_gen(1, 0)])

    drive(att3, dense3, {0: open_dense3})

    # S4: final half-norm on 4 distinct PSUM slots (attention is done, so
    # the av slots are free — no write-after-read serialization), then the
    # chunk-1 projections.
    norm_half(1, 1, rb_tags=(("av0", 1), ("av1", 1), ("mm", 2), ("mm", 2)))
    run_dense([oproj_gen(1)])
    run_dense([cproj_gen(0, 1), cproj_gen(1, 1)])
    p_nrm2.release()
    p_esc2.release()
    p_out.release()
    p_z.release()
    p_wc.release()
    p_w3.release()
    p_v.release()
    p_qk.release()
    p_y.release()
    p_den.release()
    psum.release()
    consts.release()


def _build():
    if "nc" in _CACHE:
        return _CACHE["nc"]
    nc = bacc.Bacc("TRN2", target_bir_lowering=False, debug=False,
                   enable_asserts=True, num_devices=8)
    d = nc.dram_tensor
    aps = [
        d("xT", [E, T], BF16, kind="ExternalInput").ap(),
        d("wqkT", [E, JQK], BF16, kind="ExternalInput").ap(),
        d("wvT", [E, E], BF16, kind="ExternalInput").ap(),
        d("bqk", [128, JQK // 128], F32, kind="ExternalInput").ap(),
        d("woutT", [E, E], BF16, kind="ExternalInput").ap(),
        d("wcT", [E, E], BF16, kind="ExternalInput").ap(),
        d("bc", [128, E // 128], F32, kind="ExternalInput").ap(),
        d("mask01", [128, 128], BF16, kind="ExternalInput").ap(),
        d("ind", [8, (H // 4) * 128], F32R, kind="ExternalInput").ap(),
        d("onesbf", [128, H], BF16, kind="ExternalInput").ap(),
        d("outT", [E, T], F32, kind="ExternalOutput").ap(),
    ]
    with tile.TileContext(nc) as tc:
        _emit(nc, tc, aps)
    nc.compile()
    _CACHE["nc"] = nc
    return nc


def _host_inputs(x, in_proj_w, in_proj_b, out_proj_w, out_proj_b,
                 c_proj_w, c_proj_b):
    f = np.float32
    bf = ml_dtypes.bfloat16
    x = np.asarray(x, f)
    in_proj_w = np.asarray(in_proj_w, f)
    in_proj_b = np.asarray(in_proj_b, f)
    out_proj_w = np.asarray(out_proj_w, f)
    out_proj_b = np.asarray(out_proj_b, f)
    c_proj_w = np.asarray(c_proj_w, f)
    c_proj_b = np.asarray(c_proj_b, f)
    # exact algebraic folds: v-bias and out_proj bias ride into c_proj's bias
    #   z = Wout y + (opb + Wout bv);  out = Wc z + cpb
    #   => out = Wc (Wout y) + [cpb + Wc (opb + Wout bv)]
    bout_eff = out_proj_b + out_proj_w @ in_proj_b[JQK:]
    bc_eff = c_proj_b + c_proj_w @ bout_eff
    # indicator for the denominator partition-broadcast:
    # ind[k, ap*128+j] = 1 iff k == 2*ap + j//64   (per half of 8 heads)
    ind = np.zeros((8, (H // 4) * 128), f)
    for ap in range(H // 4):
        ind[2 * ap, ap * 128:ap * 128 + 64] = 1.0
        ind[2 * ap + 1, ap * 128 + 64:(ap + 1) * 128] = 1.0
    shared = {
        "wqkT": np.ascontiguousarray(in_proj_w[:JQK].T).astype(bf),
        "wvT": np.ascontiguousarray(in_proj_w[JQK:].T).astype(bf),
        "bqk": np.ascontiguousarray(in_proj_b[:JQK].reshape(JQK // 128, 128).T),
        "woutT": np.ascontiguousarray(out_proj_w.T).astype(bf),
        "wcT": np.ascontiguousarray(c_proj_w.T).astype(bf),
        "bc": np.ascontiguousarray(bc_eff.reshape(E // 128, 128).T),
        "mask01": np.where(np.arange(128)[None, :] >= np.arange(128)[:, None],
                           f(1.0), f(0.0)).astype(bf),
        "ind": ind,
        "onesbf": np.ones((128, H), bf),
    }
    return [{**shared, "xT": np.ascontiguousarray(x[b].T).astype(bf)}
            for b in range(B)]


def kernel(x, in_proj_w, in_proj_b, out_proj_w, out_proj_b, c_proj_w,
           c_proj_b):
    nc = _build()
    in_maps = _host_inputs(x, in_proj_w, in_proj_b, out_proj_w, out_proj_b,
                           c_proj_w, c_proj_b)
    res = run_bass_kernel_spmd(nc, in_maps, core_ids=list(range(B)),
                               trace=TRACE)
    _CACHE["last_result"] = res
    out = np.stack([res.results[b]["outT"].T for b in range(B)])
    return np.ascontiguousarray(out, dtype=np.float32)
